# revision 8
# baseline (speedup 1.0000x reference)
"""Distributed Trainium2 kernel for nn_Attention_88046829568004.

Sharding (8 cores): core i handles batch-group g = i//4 and heads
{2*(i%4), 2*(i%4)+1}. Each core computes qkv for its 2 heads over its
group's 2304 tokens, full attention for those heads, then an AllGather of
normalized attention outputs within each 4-core group, and finally the
output projection for one batch (576 tokens) selected via a per-core
dynamic token offset. Host only shards inputs / concatenates outputs.

Compute dtype: bf16 on the TensorEngine (fp32 PSUM accumulation), exp in
fp32 on the ScalarEngine reading PSUM directly.
"""

import numpy as np

import concourse.bass as bass
import concourse.tile as tile
from concourse import bacc, mybir
from concourse.bass import ds
from concourse.bass_utils import run_bass_kernel_spmd

V_NUM, NUM_HEADS = 4, 8
B, C, H, W = 8, 512, 24, 24
N = V_NUM * H * W          # 2304 tokens per group
NT = H * W                 # 576 tokens per batch
HD = C // NUM_HEADS        # 64
SCALE = HD ** -0.5
CT = C // 128              # 4 contraction tiles
KC = N // 128              # 18 key chunks
QTS = [512, 512, 512, 512, 256]   # query tiles
N_CORES = 8

LAST_EXEC_NS = None


def _build():
    dt = mybir.dt
    nc = bacc.Bacc(None)

    x_g = nc.declare_dram_parameter("x_g", [C, N], dt.float32, isOutput=False)
    wqkv = nc.declare_dram_parameter("wqkv", [C, 384], dt.float32, isOutput=False)
    wproj = nc.declare_dram_parameter("wproj", [C, C], dt.float32, isOutput=False)
    bproj = nc.declare_dram_parameter("bproj", [C, 1], dt.float32, isOutput=False)
    tok0 = nc.declare_dram_parameter("tok0", [1, 1], dt.uint32, isOutput=False)
    out = nc.declare_dram_parameter("out", [C, NT], dt.float32, isOutput=True)

    ag_in = nc.dram_tensor("ag_in", [128, N], dt.bfloat16)
    ag_out = nc.dram_tensor("ag_out", [C, N], dt.bfloat16)
    den_dram = nc.dram_tensor("den_dram", [2, 64, N // 64], dt.float32)
    r_dram = nc.dram_tensor("r_dram", [2, 64, N // 64], dt.bfloat16)

    with tile.TileContext(nc) as tc:
        with (
            tc.tile_pool(name="singles", bufs=1) as singles,
            tc.tile_pool(name="stage", bufs=3) as stage,
            tc.tile_pool(name="work", bufs=3) as work,
            tc.tile_pool(name="ps", bufs=2, space="PSUM") as ps,
            tc.tile_pool(name="pso", bufs=2, space="PSUM") as pso,
        ):
            # ---- persistent SBUF tiles ----
            x_bf = singles.tile([128, CT, N], dt.bfloat16)
            wqkv_bf = singles.tile([128, CT, 384], dt.bfloat16)
            wp_bf = singles.tile([128, CT, C], dt.bfloat16)
            b_sb = singles.tile([128, CT, 1], dt.float32)
            q_pair = singles.tile([128, N], dt.bfloat16)
            k_pair = singles.tile([128, N], dt.bfloat16)
            vt_sb = singles.tile([128, KC, 130], dt.bfloat16)
            o_un = singles.tile([65, 2, N], dt.float32)
            o_bf = singles.tile([64, 2, N], dt.bfloat16)
            r_sb = singles.tile([1, 2, N], dt.bfloat16)
            ones_sb = singles.tile([1, 64], dt.bfloat16)
            tok_sb = singles.tile([1, 1], dt.uint32)
            warm_in = singles.tile([128, 1], dt.float32)
            warm_out = singles.tile([128, 1], dt.float32)

            # exp table preload (overlaps input DMA)
            nc.vector.memset(warm_in[:], 0.0)
            nc.scalar.activation(
                warm_out[:], warm_in[:], mybir.ActivationFunctionType.Exp
            )
            nc.gpsimd.memset(ones_sb[:], 1.0)
            # ones columns of [VT | 1] weight blocks (col 64 per 65-block)
            nc.gpsimd.memset(vt_sb[:, :, 64:65], 1.0)
            nc.gpsimd.memset(vt_sb[:, :, 129:130], 1.0)
            nc.sync.dma_start(out=tok_sb[:], in_=tok0[:])

            # ---- weights load + cast ----
            for ct in range(CT):
                wqf = stage.tile([128, 384], dt.float32, tag="wqs")
                nc.sync.dma_start(
                    out=wqf[:], in_=wqkv[ct * 128:(ct + 1) * 128, :]
                )
                nc.vector.tensor_copy(wqkv_bf[:, ct, :], wqf[:])
                wpf = stage.tile([128, C], dt.float32, tag="wps")
                nc.sync.dma_start(
                    out=wpf[:], in_=wproj[ct * 128:(ct + 1) * 128, :]
                )
                nc.vector.tensor_copy(wp_bf[:, ct, :], wpf[:])
                nc.sync.dma_start(
                    out=b_sb[:, ct, :], in_=bproj[ct * 128:(ct + 1) * 128, :]
                )

            # ---- X load + cast (by token slice, so qkv can start early) ----
            for tt in range(len(QTS)):
                t0, tw = tt * 512, QTS[tt]
                xf = stage.tile([128, CT, 512], dt.float32, tag="xstage")
                for ct in range(CT):
                    nc.sync.dma_start(
                        out=xf[:, ct, :tw],
                        in_=x_g[ct * 128:(ct + 1) * 128, t0:t0 + tw],
                    )
                    nc.vector.tensor_copy(
                        x_bf[:, ct, t0:t0 + tw], xf[:, ct, :tw]
                    )

            # ---- qkv: Q and K chunks (rows 0-63 head0, 64-127 head1) ----
            for tt in range(len(QTS)):
                t0, tw = tt * 512, QTS[tt]
                for oc in range(2):  # 0 = Q, 1 = K
                    pq = ps.tile([128, 3, 512], dt.float32, tag="s")
                    for ct in range(CT):
                        nc.tensor.matmul(
                            pq[:, 0, :tw],
                            lhsT=wqkv_bf[:, ct, oc * 128:(oc + 1) * 128],
                            rhs=x_bf[:, ct, t0:t0 + tw],
                            start=(ct == 0),
                            stop=(ct == CT - 1),
                        )
                    dst = q_pair if oc == 0 else k_pair
                    nc.vector.tensor_copy(dst[:, t0:t0 + tw], pq[:, 0, :tw])

            # ---- V^T (keys on partitions): psum[t_chunk, v_cols] ----
            for kc in range(KC):
                pv = ps.tile([128, 3, 512], dt.float32, tag="s")
                for ct in range(CT):
                    nc.tensor.matmul(
                        pv[:, 0, 0:128],
                        lhsT=x_bf[:, ct, kc * 128:(kc + 1) * 128],
                        rhs=wqkv_bf[:, ct, 256:384],
                        start=(ct == 0),
                        stop=(ct == CT - 1),
                    )
                nc.vector.tensor_copy(
                    vt_sb[:, kc, :].rearrange("p (h y) -> p h y", h=2)[:, :, 0:64],
                    pv[:, 0, 0:128].rearrange("p (h y) -> p h y", h=2),
                )

            # ---- attention: S = K^T Q (row-packed heads), exp, O = [V^T|1] P ----
            for qt in range(len(QTS)):
                q0, qtw = qt * 512, QTS[qt]
                po = [
                    pso.tile([65, 512], dt.float32, tag="o", name=f"po{h}")
                    for h in range(2)
                ]
                for t in range(KC * 2 // 3):  # 12 psum-triples per q tile
                    pst = ps.tile([128, 3, 512], dt.float32, tag="s")
                    trip = []
                    for s in range(3):
                        idx = t * 3 + s
                        kc, h = idx // 2, idx % 2
                        nc.tensor.matmul(
                            pst[:, s, :qtw],
                            lhsT=k_pair[h * 64:(h + 1) * 64, kc * 128:(kc + 1) * 128],
                            rhs=q_pair[h * 64:(h + 1) * 64, q0:q0 + qtw],
                            start=True,
                            stop=True,
                            tile_position=(h * 64, 0),
                        )
                        trip.append((kc, h, s))
                    pt = work.tile([128, 3, 512], dt.bfloat16, tag="p")
                    nc.scalar.activation(
                        pt[:, :, :qtw],
                        pst[:, :, :qtw],
                        mybir.ActivationFunctionType.Exp,
                        scale=SCALE,
                    )
                    for kc, h, s in trip:
                        nc.tensor.matmul(
                            po[h][:, :qtw],
                            lhsT=vt_sb[:, kc, h * 65:(h + 1) * 65],
                            rhs=pt[:, s, :qtw],
                            start=(kc == 0),
                            stop=(kc == KC - 1),
                        )
                for h in range(2):
                    nc.vector.tensor_copy(
                        o_un[:, h, q0:q0 + qtw], po[h][:, :qtw]
                    )

            # ---- softmax denominators -> reciprocal (partition-parallel) ----
            for h in range(2):
                nc.sync.dma_start(
                    out=den_dram[h], in_=o_un[64:65, h, :]
                )
            den128 = work.tile([128, N // 64], dt.float32, tag="den")
            for h in range(2):
                nc.sync.dma_start(
                    out=den128[h * 64:(h + 1) * 64, :], in_=den_dram[h]
                )
            r128 = work.tile([128, N // 64], dt.float32, tag="rden")
            nc.vector.reciprocal(r128[:], den128[:])
            r128_bf = work.tile([128, N // 64], dt.bfloat16, tag="rbf")
            nc.vector.tensor_copy(r128_bf[:], r128[:])
            for h in range(2):
                nc.sync.dma_start(
                    out=r_dram[h], in_=r128_bf[h * 64:(h + 1) * 64, :]
                )
            nc.sync.dma_start(out=r_sb[:], in_=r_dram[:])

            # ---- normalize O (broadcast 1/den via K=1 matmul) + stage AG input ----
            for qt in range(len(QTS)):
                q0, qtw = qt * 512, QTS[qt]
                for h in range(2):
                    pb = ps.tile([64, 512], dt.float32, tag="s")
                    nc.tensor.matmul(
                        pb[:, :qtw],
                        lhsT=ones_sb[:],
                        rhs=r_sb[0:1, h, q0:q0 + qtw],
                        start=True,
                        stop=True,
                    )
                    nc.vector.tensor_mul(
                        o_bf[:, h, q0:q0 + qtw],
                        o_un[0:64, h, q0:q0 + qtw],
                        pb[:, :qtw],
                    )
                    nc.gpsimd.dma_start(
                        out=ag_in[h * 64:(h + 1) * 64, q0:q0 + qtw],
                        in_=o_bf[:, h, q0:q0 + qtw],
                    )

            # ---- AllGather O within each 4-core group ----
            import os as _os
            if _os.environ.get("KERNEL_NO_COLLECTIVE"):
                for rr in range(4):
                    nc.gpsimd.dma_start(
                        out=ag_out[rr * 128:(rr + 1) * 128, :], in_=ag_in[:]
                    )
            else:
                nc.gpsimd.collective_compute(
                    "AllGather",
                    mybir.AluOpType.bypass,
                    ins=[ag_in[:]],
                    outs=[ag_out[:]],
                    replica_groups=[[0, 1, 2, 3], [4, 5, 6, 7]],
                )

            # ---- fetch my batch's tokens (dynamic offset) ----
            og_sb = singles.tile([128, CT, NT], dt.bfloat16)
            if _os.environ.get("KERNEL_STATIC_OFF"):
                for ct in range(CT):
                    nc.sync.dma_start(
                        out=og_sb[:, ct, :],
                        in_=ag_out[ct * 128:(ct + 1) * 128, 0:NT],
                    )
            else:
                rg = nc.sync.alloc_register("tok0reg")
                nc.sync.reg_load(rg, tok_sb[0:1, 0:1])
                sv = nc.sync.snap(
                    rg, donate=True, min_val=0, max_val=(V_NUM - 1) * NT
                )
                for ct in range(CT):
                    nc.sync.dma_start(
                        out=og_sb[:, ct, :],
                        in_=ag_out[ct * 128:(ct + 1) * 128, ds(sv, NT)],
                    )

            # ---- projection + bias ----
            for oc in range(CT):
                for tb, tw2 in ((0, 512), (512, NT - 512)):
                    py_ = ps.tile([128, 3, 512], dt.float32, tag="s")
                    for ct in range(CT):
                        nc.tensor.matmul(
                            py_[:, 0, :tw2],
                            lhsT=wp_bf[:, ct, oc * 128:(oc + 1) * 128],
                            rhs=og_sb[:, ct, tb:tb + tw2],
                            start=(ct == 0),
                            stop=(ct == CT - 1),
                        )
                    ysb = work.tile([128, 512], dt.float32, tag="y")
                    nc.vector.tensor_scalar_add(
                        ysb[:, :tw2], py_[:, 0, :tw2], b_sb[:, oc, :]
                    )
                    nc.sync.dma_start(
                        out=out[oc * 128:(oc + 1) * 128, tb:tb + tw2],
                        in_=ysb[:, :tw2],
                    )

    nc.compile()
    return nc


_NC_CACHE = None


def _get_nc():
    global _NC_CACHE
    if _NC_CACHE is None:
        _NC_CACHE = _build()
    return _NC_CACHE


def build_in_maps(inputs):
    return _build_in_maps(
        inputs["x"], inputs["w_qkv"], inputs["w_proj"], inputs["b_proj"]
    )


def _build_in_maps(x, w_qkv, w_proj, b_proj):
    x = np.ascontiguousarray(np.asarray(x, dtype=np.float32))
    w_qkv = np.asarray(w_qkv, dtype=np.float32)
    w_proj = np.asarray(w_proj, dtype=np.float32)
    b_proj = np.asarray(b_proj, dtype=np.float32)
    wprojT = np.ascontiguousarray(w_proj.T)
    bcol = np.ascontiguousarray(b_proj.reshape(C, 1))
    in_maps = []
    for i in range(N_CORES):
        g, r = divmod(i, 4)
        h0 = 2 * r
        xg = np.ascontiguousarray(
            x[g * V_NUM:(g + 1) * V_NUM].transpose(1, 0, 2, 3).reshape(C, N)
        )
        rows = np.concatenate(
            [
                w_qkv[h0 * 192 + 0:h0 * 192 + 64],
                w_qkv[(h0 + 1) * 192 + 0:(h0 + 1) * 192 + 64],
                w_qkv[h0 * 192 + 64:h0 * 192 + 128],
                w_qkv[(h0 + 1) * 192 + 64:(h0 + 1) * 192 + 128],
                w_qkv[h0 * 192 + 128:h0 * 192 + 192],
                w_qkv[(h0 + 1) * 192 + 128:(h0 + 1) * 192 + 192],
            ],
            axis=0,
        )
        in_maps.append(
            {
                "x_g": xg,
                "wqkv": np.ascontiguousarray(rows.T),
                "wproj": wprojT,
                "bproj": bcol,
                "tok0": np.array([[r * NT]], dtype=np.uint32),
            }
        )
    return in_maps


def kernel(x, w_qkv, w_proj, b_proj):
    global LAST_EXEC_NS
    nc = _get_nc()
    in_maps = _build_in_maps(x, w_qkv, w_proj, b_proj)
    res = run_bass_kernel_spmd(nc, in_maps, core_ids=list(range(N_CORES)))
    LAST_EXEC_NS = res.exec_time_ns

    out = np.empty((B, C, H, W), dtype=np.float32)
    for i in range(N_CORES):
        g, r = divmod(i, 4)
        out[g * V_NUM + r] = res.results[i]["out"].reshape(C, H, W)
    return out
